# revision 5
# baseline (speedup 1.0000x reference)
"""Trainium2 Bass kernel for nn_LogicLayer (soft logic-gate layer).

Math (per core, batch-sharded):
  pA = softmax(Wa, axis=1); pB = softmax(Wb, axis=1); pT = softmax(tw, axis=0)
  a = pA @ X ; b = pB @ X
  out = sum_g pT[g] * gate_g(a, b)

Each of the 16 soft gates is affine in {1, A, B, A*B}, so with C[g, :] =
(c1, cA, cB, cAB) per gate:
  out = w1 + wA*a + wB*b + wAB*(a*b),   w_j[m] = sum_g pT[g, m] * C[g, j]

All softmax normalizers fold into the coefficients: with unnormalized
Ea = exp(Wa) (no max-subtraction needed; Wa ~ N(0,1)), ta = Ea^T-matmul,
a = ta / sA[m], and pT = exp(tw)/sT:
  out = w1' + wA'*ta + wB'*tb + wAB'*ta*tb
  w1' = w1raw/sT, wA' = wAraw/(sT*sA), wB' = wBraw/(sT*sB),
  wAB' = wABraw/(sT*sA*sB)

Device pipeline (per core; batch 16384 sharded 8 ways -> NB=2048):
  1. Load Wa^T, Wb^T (host-staged transposed layout), exp -> bf16 EaT/EbT.
  2. Row sums sA[m] via tiny N=1 PE matmuls against a ones vector.
  3. exp(tw) f32; one [16,5] constant matmul gives all 4 raw coefficient
     combos + sT; small DVE ops assemble final per-row coefficients.
  4. Main loop: 2 bf16 matmul groups (K=1024) per (m-block, n-tile 512),
     fused epilogue on DVE/ACT/GPSIMD, DMA out.
"""

import sys

if "/opt/trn_rl_repo" not in sys.path:
    sys.path.insert(0, "/opt/trn_rl_repo")

import numpy as np

import concourse.bass as bass
import concourse.mybir as mybir
import concourse.tile as tile
from concourse.bass_utils import run_bass_kernel_spmd

N_CORES = 8
SIZE = 1024
PREV = 1024
BATCH = 16384
NB = BATCH // N_CORES  # 2048 batch columns per core
NT = 512               # n-tile (one PSUM bank of f32)
N_NT = NB // NT        # 4
KB = PREV // 128       # 8 k-blocks
MB = SIZE // 128       # 8 m-blocks

F32 = mybir.dt.float32
BF16 = mybir.dt.bfloat16
FP8 = mybir.dt.float8e4

# fp8e4m3 + DoubleRow: ~1.5x PE throughput on the two big matmuls at the
# cost of ~2.5e-3 max rel err (CPU-sim; bf16 gives ~2e-4).
USE_FP8 = False

# Gate coefficient matrix: columns = [const, A, B, AB, ones]; rows = gate id.
_C16 = np.array(
    [
        # 1   A   B  AB  ones
        [0,  0,  0,  0, 1],  # 0  FALSE
        [0,  0,  0,  1, 1],  # 1  A AND B
        [0,  1,  0, -1, 1],  # 2  A AND NOT B
        [0,  1,  0,  0, 1],  # 3  A
        [0,  0,  1, -1, 1],  # 4  NOT A AND B
        [0,  0,  1,  0, 1],  # 5  B
        [0,  1,  1, -2, 1],  # 6  XOR
        [0,  1,  1, -1, 1],  # 7  OR
        [1, -1, -1,  1, 1],  # 8  NOR
        [1, -1, -1,  2, 1],  # 9  XNOR
        [1,  0, -1,  0, 1],  # 10 NOT B
        [1,  0, -1,  1, 1],  # 11 B -> A
        [1, -1,  0,  0, 1],  # 12 NOT A
        [1, -1,  0,  1, 1],  # 13 A -> B
        [1,  0,  0, -1, 1],  # 14 NAND
        [1,  0,  0,  0, 1],  # 15 TRUE
    ],
    dtype=np.float32,
)


def _split_waits(nc, maxw=1):
    """Walrus in this container encodes at most one sync-wait per
    instruction; hoist excess waits into preceding NoOps on the same
    engine (semantically an AND of waits, executed in sequence)."""
    for f in nc.m.functions:
        for blk in f.blocks:
            new_list = []
            changed = False
            for inst in blk.instructions:
                si = inst.sync_info
                if si is not None and len(si.on_wait) > maxw:
                    waits = list(si.on_wait)
                    chunks = [waits[i : i + maxw] for i in range(0, len(waits), maxw)]
                    for ci, ch in enumerate(chunks[:-1]):
                        nop = mybir.InstNoOp(
                            name=f"{inst.name}-wsplit{ci}", ins=[], outs=[]
                        )
                        nop.engine = inst.engine
                        nop.sync_info = mybir.SyncInfo(on_wait=ch, on_update=[])
                        new_list.append(nop)
                    inst.sync_info = mybir.SyncInfo(
                        on_wait=chunks[-1], on_update=list(si.on_update)
                    )
                    changed = True
                new_list.append(inst)
            if changed:
                blk.instructions = new_list


def build_nc():
    nc = bass.Bass()
    x_d = nc.dram_tensor("x", [PREV, NB], F32, kind="ExternalInput")
    wat_d = nc.dram_tensor("wat", [PREV, SIZE], F32, kind="ExternalInput")
    wbt_d = nc.dram_tensor("wbt", [PREV, SIZE], F32, kind="ExternalInput")
    tw_d = nc.dram_tensor("tw", [16, SIZE], F32, kind="ExternalInput")
    out_d = nc.dram_tensor("out", [SIZE, NB], F32, kind="ExternalOutput")
    c16_d = nc.inline_tensor(_C16, "c16")

    AF = mybir.ActivationFunctionType
    OP = mybir.AluOpType

    with tile.TileContext(nc) as tc:
        with (
            tc.tile_pool(name="persist", bufs=1) as pp,
            tc.tile_pool(name="wstage", bufs=3) as wstage,
            tc.tile_pool(name="xstage", bufs=6) as xstage,
            tc.tile_pool(name="xbuf", bufs=2) as xbuf,
            tc.tile_pool(name="epi", bufs=3) as epi,
            tc.tile_pool(name="outp", bufs=4) as outp,
            tc.tile_pool(name="psum", bufs=2, space="PSUM") as psp,
            tc.tile_pool(name="psum1", bufs=1, space="PSUM") as psp1,
        ):
            # --- constants ---
            c16s = pp.tile([16, 5], F32, tag="c16s", name="c16s")
            nc.sync.dma_start(out=c16s, in_=c16_d[:, :])
            ones = pp.tile([128, 1], BF16, tag="ones", name="ones")
            nc.vector.memset(ones, 1.0)

            # --- table coefficients ---
            tws = pp.tile([16, SIZE], F32, tag="tws", name="tws")
            nc.sync.dma_start(out=tws, in_=tw_d[:, :])
            et = pp.tile([16, SIZE], F32, tag="et", name="et")
            nc.scalar.activation(et, tws, AF.Exp)
            # fp32 PE matmuls only carry ~bf16 precision here, so split et
            # into bf16 hi+lo and accumulate two exact bf16 matmuls.
            c16b = pp.tile([16, 5], BF16, tag="c16b", name="c16b")
            nc.vector.tensor_copy(c16b, c16s)
            ethi = pp.tile([16, SIZE], BF16, tag="ethi", name="ethi")
            nc.vector.tensor_copy(ethi, et)
            etlo = pp.tile([16, SIZE], BF16, tag="etlo", name="etlo")
            nc.vector.scalar_tensor_tensor(
                etlo, et, 1.0, ethi, op0=OP.mult, op1=OP.subtract
            )
            psw = psp1.tile([128, MB, 5], F32, tag="psw", name="psw")
            for mb in range(MB):
                ms = slice(mb * 128, (mb + 1) * 128)
                nc.tensor.matmul(
                    psw[:, mb, :], ethi[:, ms], c16b[:, :], start=True, stop=False
                )
                nc.tensor.matmul(
                    psw[:, mb, :], etlo[:, ms], c16b[:, :], start=False, stop=True
                )

            # --- weights: exp in transposed layout + row sums ---
            eaT = [pp.tile([128, SIZE], BF16, tag=f"ea{kb}", name=f"ea{kb}") for kb in range(KB)]
            ebT = [pp.tile([128, SIZE], BF16, tag=f"eb{kb}", name=f"eb{kb}") for kb in range(KB)]
            pssa = psp1.tile([128, MB], F32, tag="pssa", name="pssa")
            pssb = psp1.tile([128, MB], F32, tag="pssb", name="pssb")
            for kb in range(KB):
                ks = slice(kb * 128, (kb + 1) * 128)
                wfa = wstage.tile([128, SIZE], F32, tag="wf32", name="wf32")
                nc.sync.dma_start(out=wfa, in_=wat_d[ks, :])
                nc.scalar.activation(eaT[kb], wfa, AF.Exp)
                wfb = wstage.tile([128, SIZE], F32, tag="wf32", name="wf32")
                nc.sync.dma_start(out=wfb, in_=wbt_d[ks, :])
                nc.scalar.activation(ebT[kb], wfb, AF.Exp)
            # mb-outer so each column's PSUM accumulation group is contiguous
            # in PE order — interleaved groups in one bank corrupt results.
            for mb in range(MB):
                ms = slice(mb * 128, (mb + 1) * 128)
                for kb in range(KB):
                    nc.tensor.matmul(
                        pssa[:, mb : mb + 1],
                        eaT[kb][:, ms],
                        ones[:, :],
                        start=(kb == 0),
                        stop=(kb == KB - 1),
                    )
                for kb in range(KB):
                    nc.tensor.matmul(
                        pssb[:, mb : mb + 1],
                        ebT[kb][:, ms],
                        ones[:, :],
                        start=(kb == 0),
                        stop=(kb == KB - 1),
                    )

            # --- assemble final coefficients [128, MB] ---
            sa = pp.tile([128, MB], F32, tag="sa", name="sa")
            nc.vector.tensor_copy(sa, pssa)
            sb = pp.tile([128, MB], F32, tag="sb", name="sb")
            nc.vector.tensor_copy(sb, pssb)
            ra = pp.tile([128, MB], F32, tag="ra", name="ra")
            nc.vector.reciprocal(ra, sa)
            rb = pp.tile([128, MB], F32, tag="rb", name="rb")
            nc.vector.reciprocal(rb, sb)
            wraw = pp.tile([128, MB, 5], F32, tag="wraw", name="wraw")
            nc.vector.tensor_copy(wraw, psw)
            rt = pp.tile([128, MB], F32, tag="rt", name="rt")
            nc.vector.reciprocal(rt, wraw[:, :, 4])
            tA = pp.tile([128, MB], F32, tag="tA", name="tA")
            nc.vector.tensor_mul(tA, rt, ra)
            tB = pp.tile([128, MB], F32, tag="tB", name="tB")
            nc.vector.tensor_mul(tB, rt, rb)
            tAB = pp.tile([128, MB], F32, tag="tAB", name="tAB")
            nc.vector.tensor_mul(tAB, tA, rb)
            w1f = pp.tile([128, MB], F32, tag="w1f", name="w1f")
            nc.vector.tensor_mul(w1f, wraw[:, :, 0], rt)
            wAf = pp.tile([128, MB], F32, tag="wAf", name="wAf")
            nc.vector.tensor_mul(wAf, wraw[:, :, 1], tA)
            wBf = pp.tile([128, MB], F32, tag="wBf", name="wBf")
            nc.vector.tensor_mul(wBf, wraw[:, :, 2], tB)
            wABf = pp.tile([128, MB], F32, tag="wABf", name="wABf")
            nc.vector.tensor_mul(wABf, wraw[:, :, 3], tAB)

            # --- main loop ---
            for nt in range(N_NT):
                ns = slice(nt * NT, (nt + 1) * NT)
                xb = []
                for kb in range(KB):
                    ks = slice(kb * 128, (kb + 1) * 128)
                    xf = xstage.tile([128, NT], F32, tag="xf", name="xf")
                    nc.sync.dma_start(out=xf, in_=x_d[ks, ns])
                    xbt = xbuf.tile([128, NT], BF16, tag=f"xb{kb}", name=f"xb{kb}")
                    nc.scalar.activation(xbt, xf, AF.Copy)
                    xb.append(xbt)
                for mb in range(MB):
                    ms = slice(mb * 128, (mb + 1) * 128)
                    pa = psp.tile([128, NT], F32, tag="pa", name="pa")
                    pb = psp.tile([128, NT], F32, tag="pb", name="pb")
                    for kb in range(KB):
                        nc.tensor.matmul(
                            pa,
                            eaT[kb][:, ms],
                            xb[kb][:, :],
                            start=(kb == 0),
                            stop=(kb == KB - 1),
                        )
                    for kb in range(KB):
                        nc.tensor.matmul(
                            pb,
                            ebT[kb][:, ms],
                            xb[kb][:, :],
                            start=(kb == 0),
                            stop=(kb == KB - 1),
                        )
                    # epilogue: out = (ta*wAB' + wA')*ta? -- no:
                    #   u = tb*wAB' + wA'          (DVE tensor_scalar dual-op)
                    #   v = tb*wB' + w1'           (ACT identity scale/bias)
                    #   w = ta*u                   (DVE)
                    #   o = w + v                  (GPSIMD, SBUF only)
                    u = epi.tile([128, NT], F32, tag="u", name="u")
                    nc.vector.tensor_scalar(
                        u,
                        pb,
                        wABf[:, mb : mb + 1],
                        wAf[:, mb : mb + 1],
                        op0=OP.mult,
                        op1=OP.add,
                    )
                    v = epi.tile([128, NT], F32, tag="v", name="v")
                    nc.scalar.activation(
                        v,
                        pb,
                        AF.Identity,
                        bias=w1f[:, mb : mb + 1],
                        scale=wBf[:, mb : mb + 1],
                    )
                    w = epi.tile([128, NT], F32, tag="w", name="w")
                    nc.vector.tensor_mul(w, pa, u)
                    o = outp.tile([128, NT], F32, tag="o", name="o")
                    nc.gpsimd.tensor_add(o, w, v)
                    nc.sync.dma_start(out=out_d[ms, ns], in_=o)

    _split_waits(nc)
    return nc


_NC_CACHE = None


def _get_nc():
    global _NC_CACHE
    if _NC_CACHE is None:
        _NC_CACHE = build_nc()
    return _NC_CACHE


def kernel(prev_layer_output, input_A_weights, input_B_weights, table_weights):
    x = np.ascontiguousarray(np.asarray(prev_layer_output, dtype=np.float32))
    wa = np.asarray(input_A_weights, dtype=np.float32)
    wb = np.asarray(input_B_weights, dtype=np.float32)
    tw = np.ascontiguousarray(np.asarray(table_weights, dtype=np.float32))
    wat = np.ascontiguousarray(wa.T)
    wbt = np.ascontiguousarray(wb.T)

    nc = _get_nc()
    in_maps = [
        {
            "x": np.ascontiguousarray(x[:, c * NB : (c + 1) * NB]),
            "wat": wat,
            "wbt": wbt,
            "tw": tw,
        }
        for c in range(N_CORES)
    ]
    res = run_bass_kernel_spmd(nc, in_maps, core_ids=list(range(N_CORES)))
    return np.concatenate([res.results[c]["out"] for c in range(N_CORES)], axis=1)
